# revision 127
# baseline (speedup 1.0000x reference)
"""Trainium2 Bass kernel for nn_Block (causal attention + noisy top-2 MoE).

Sharding (SPMD, 8 cores, identical program - only input data differs/core):
- Attention head-sharded: core c computes heads {2c, 2c+1} for all 2048
  tokens (w_qkv column slice + w_proj row slice as per-core inputs), then a
  ReduceScatter(add) of the partial projection output token-shards x_attn.
- MoE expert-parallel: core c owns expert c (w1/w2 slices as inputs).
  Router computed per token-shard, logits AllGathered, routing replicated,
  normalized activations AllGathered in bf16, expert's tokens fetched via
  indirect-DMA row gather, FFN (SBUF-resident bf16 weights), outputs
  scattered to a token-indexed combine buffer, ReduceScatter(add) in bf16,
  final residual add.

LayerNorm gamma/beta are folded into the consuming weight matrices on the
host, so the kernel only materializes the normalized (pre-gamma) tensor.
"""
import math
import ml_dtypes
import numpy as np

import concourse.bass as bass
import concourse.mybir as mybir
import concourse.tile as tile
from concourse.bass import IndirectOffsetOnAxis
from concourse.bass_utils import run_bass_kernel_spmd
from concourse.masks import make_identity

F32 = mybir.dt.float32
F32R = mybir.dt.float32r
BF16 = mybir.dt.bfloat16
I32 = mybir.dt.int32
AX = mybir.AxisListType
ALU = mybir.AluOpType
ACTF = mybir.ActivationFunctionType

B, T, D, H = 2, 1024, 1024, 16
NEXP, TOPK = 8, 2
DH = D // H          # 64
HALF = DH // 2       # 32
DFF = 4 * D          # 4096
NTOK = B * T         # 2048
CAP = NTOK * TOPK // NEXP  # 512
NC = 8
LT = NTOK // NC      # 256 local tokens per core
NT = NTOK // 128     # 16 global token tiles
NT_LOC = LT // 128   # 2


def split_multiwaits(nc):
    """This walrus encodes ONE sem wait per instruction; split extras into
    single-wait NOPs preceding the instruction on the same engine."""
    n = 0
    for f in nc.m.functions:
        for bb in f.blocks:
            new = []
            changed = False
            for ins in bb.instructions:
                si = ins.sync_info
                if si is not None and len(si.on_wait) > 1:
                    waits = list(si.on_wait)
                    for w in waits[:-1]:
                        new.append(mybir.InstNoOp(
                            name=f"I-{nc.next_id()}", engine=ins.engine,
                            ins=[], outs=[],
                            sync_info=mybir.SyncInfo(on_wait=[w], on_update=[]),
                            bass_nofuse=True))
                        n += 1
                    ins.sync_info = mybir.SyncInfo(
                        on_wait=[waits[-1]], on_update=list(si.on_update))
                    changed = True
                new.append(ins)
            if changed:
                bb.instructions = new
    return n


def build_kernel():
    nc = bass.Bass("TRN2", target_bir_lowering=False, debug=False,
                   enable_asserts=True, num_devices=NC)

    def din(name, shape, dt=F32):
        return nc.dram_tensor(name, list(shape), dt, kind="ExternalInput")

    x_d = din("x_full", (NTOK, D))
    xsl_d = din("x_slice", (LT, D))
    cos_d = din("cos_tm", (128, NT * HALF))
    sin_d = din("sin_tm", (128, NT * HALF))
    wqkv_d = din("w_qkv_l", (D, 3 * 128), F32R)
    bqkv_d = din("b_qkv_l", (1, 3 * 128))
    wproj_d = din("w_proj_l", (128, D), F32R)
    wrl_d = din("w_rlrn", (D, 16), F32R)
    brl_d = din("b_rlrn", (1, 16))
    csum_d = din("csum_rlrn", (1, 16))
    w1_d = din("w1_l", (D, DFF), BF16)
    w2_d = din("w2_l", (DFF, D), BF16)
    b1_d = din("b1_l", (128, DFF // 128))
    b2_d = din("b2_l", (128, D // 128))
    noise_d = din("noise_t", (NTOK, NEXP))
    onehot_d = din("onehot", (1, NEXP))
    rowid_d = din("rowid", (128, NT))
    pfx_d = din("pfx", (128, 128))
    sut_d = din("sut", (128, 128), F32R)
    causal_d = din("causal", (128, 128))

    out_d = nc.dram_tensor("out_c", [LT, D], F32, kind="ExternalOutput")

    prs_in = nc.dram_tensor("prs_in", [NTOK, D], F32)
    prs_out = nc.dram_tensor("prs_out", [LT, D], F32)
    h2ag_in = nc.dram_tensor("h2ag_in", [LT, D], BF16)
    h2ag = nc.dram_tensor("h2ag", [NTOK, D], BF16, addr_space="Shared")
    lgag_in = nc.dram_tensor("lgag_in", [LT, 16], F32)
    lgag = nc.dram_tensor("lgag", [NTOK, 16], F32, addr_space="Shared")
    meta_d = nc.dram_tensor("meta_d", [CAP, 2], F32)
    xmid_d = nc.dram_tensor("xmid_d", [LT, D], F32)
    comb = nc.dram_tensor("comb", [NTOK + 1, D], BF16)
    rs2_out = nc.dram_tensor("rs2_out", [LT, D], BF16)

    RG = [list(range(NC))]

    with tile.TileContext(nc) as tc:
        with (
            tc.tile_pool(name="cst", bufs=1) as cst,
            tc.tile_pool(name="scr", bufs=2) as scr,
            tc.tile_pool(name="resg", bufs=1) as resg,
        ):
            # ---------------- constants ----------------
            ident = cst.tile([128, 128], F32)
            make_identity(nc, ident[:])
            ident_r = cst.tile([128, 128], F32R)
            nc.vector.tensor_copy(ident_r[:], ident[:])
            ident_bf = cst.tile([128, 128], BF16)
            nc.vector.tensor_copy(ident_bf[:], ident[:])
            sut_t = cst.tile([128, 128], F32R)
            nc.sync.dma_start(sut_t[:], sut_d[:])
            pfx_t = cst.tile([128, 128], F32)
            nc.sync.dma_start(pfx_t[:], pfx_d[:])
            ones1f = cst.tile([1, 128], F32)
            nc.vector.memset(ones1f[:], 1.0)
            ones1 = cst.tile([1, 128], F32R)
            nc.vector.tensor_copy(ones1[:], ones1f[:])
            ones128f = cst.tile([128, 1], F32)
            nc.vector.memset(ones128f[:], 1.0)
            ones128 = cst.tile([128, 1], F32R)
            nc.vector.tensor_copy(ones128[:], ones128f[:])
            rowid_t = cst.tile([128, NT], F32)
            nc.sync.dma_start(rowid_t[:], rowid_d[:])
            b1_t = cst.tile([128, DFF // 128], F32)
            nc.sync.dma_start(b1_t[:], b1_d[:])
            b2_t = cst.tile([128, D // 128], F32)
            nc.sync.dma_start(b2_t[:], b2_d[:])
            c2048 = cst.tile([128, 1], F32)
            nc.vector.memset(c2048[:], float(NTOK))
            eps_t = cst.tile([128, 1], F32)
            nc.vector.memset(eps_t[:], 1e-5)
            neg1_t = cst.tile([128, 1], F32)
            nc.vector.memset(neg1_t[:], -1.0)
            z1_t = cst.tile([128, 1], F32)
            nc.vector.memset(z1_t[:], 0.0)
            one_t = cst.tile([128, 1], F32)
            nc.vector.memset(one_t[:], 1.0)

            def bcast_row(src_dram, w, nm):
                row = cst.tile([1, w], F32R, tag=f"bcr_{nm}")
                nc.gpsimd.dma_start(row[:], src_dram[:])
                outt = cst.tile([128, w], F32, tag=f"bcm_{nm}")
                with tc.tile_pool(name=f"psO_{nm}", bufs=1,
                                  space="PSUM") as psO:
                    pb = psO.tile([128, 512], F32, tag="pO")
                    nc.tensor.matmul(pb[:, :w], ones1[:], row[:],
                                     start=True, stop=True)
                    nc.scalar.copy(outt[:], pb[:, :w])
                return outt

            ohB = bcast_row(onehot_d, NEXP, "oh")
            csumB = bcast_row(csum_d, 16, "cs")
            brlB = bcast_row(brl_d, 16, "brl")

            # FFN w1, SBUF-resident (bf16); loads emitted after the QKV
            # phase so the DMA traffic lands in an otherwise-idle window.
            w1_sb = [cst.tile([128, DFF], BF16, tag=f"w1r{k}",
                              name=f"w1sb{k}") for k in range(8)]

            # attention-scoped residents
            p_attres_cm = tc.tile_pool(name="p_attres", bufs=1)
            p_attres = p_attres_cm.__enter__()
            # PSUM pools are phase-scoped (8 banks total); phase A set:
            psA_cm = tc.tile_pool(name="psA", bufs=4, space="PSUM")
            psA = psA_cm.__enter__()
            psB_cm = tc.tile_pool(name="psB", bufs=4, space="PSUM")
            psB = psB_cm.__enter__()
            causal_t = p_attres.tile([128, 128], F32)
            nc.sync.dma_start(causal_t[:], causal_d[:])
            causal_r = p_attres.tile([128, 128], F32R)
            nc.vector.tensor_copy(causal_r[:], causal_t[:])
            bqkv_t = p_attres.tile([1, 3 * 128], F32R)
            nc.gpsimd.dma_start(bqkv_t[:], bqkv_d[:])
            cos_t = p_attres.tile([128, NT * HALF], F32)
            nc.sync.dma_start(cos_t[:], cos_d[:])
            sin_t = p_attres.tile([128, NT * HALF], F32)
            nc.sync.dma_start(sin_t[:], sin_d[:])
            wqkv_sb = []
            for k in range(8):
                wt = p_attres.tile([128, 384], F32R, tag=f"wqkv{k}")
                nc.sync.dma_start(wt[:], wqkv_d[k * 128:(k + 1) * 128, :])
                wqkv_sb.append(wt)
            wproj_sb = []
            for n in range(2):
                wt = p_attres.tile([128, 512], F32R, tag=f"wproj{n}")
                nc.sync.dma_start(wt[:], wproj_d[:, n * 512:(n + 1) * 512])
                wproj_sb.append(wt)
            wrl_sb = []
            for k in range(8):
                wt = cst.tile([128, 16], F32R, tag=f"wrl{k}")
                nc.sync.dma_start(wt[:], wrl_d[k * 128:(k + 1) * 128, :])
                wrl_sb.append(wt)

            qT = p_attres.tile([128, NTOK], F32R, tag="qT")
            kT = p_attres.tile([128, NTOK], F32R, tag="kT")
            v_tm = []
            for t in range(NT):
                vt_ = p_attres.tile([128, 128], BF16, tag=f"v{t}",
                                    name=f"v_tm{t}")
                v_tm.append(vt_)
            aoT = p_attres.tile([128, NTOK], F32R, tag="aoT")

            # ---------------- helpers ----------------
            def ln_norm(pool, xt, out, tag):
                """out = (xt - mean)/std, row-wise over D (no gamma/beta).
                Returns the (rstd, -mean*rstd) column tiles."""
                st = pool.tile([128, 2, 6], F32, tag=f"{tag}_st")
                nc.vector.bn_stats(st[:, 0, :], xt[:, 0:512])
                nc.vector.bn_stats(st[:, 1, :], xt[:, 512:1024])
                mv = pool.tile([128, 2], F32, tag=f"{tag}_mv")
                nc.vector.bn_aggr(mv[:], st[:])
                lnv = pool.tile([128, 1], F32, tag=f"{tag}_lnv")
                nc.scalar.activation(lnv[:], mv[:, 1:2], ACTF.Ln,
                                     bias=eps_t[:, 0:1], scale=1.0)
                rstd = pool.tile([128, 1], F32, tag=f"{tag}_rstd")
                nc.scalar.activation(rstd[:], lnv[:], ACTF.Exp,
                                     bias=z1_t[:, 0:1], scale=-0.5)
                negmr = pool.tile([128, 1], F32, tag=f"{tag}_nmr")
                nc.vector.scalar_tensor_tensor(negmr[:], mv[:, 0:1], -1.0,
                                               rstd[:], op0=ALU.mult,
                                               op1=ALU.mult)
                nc.gpsimd.tensor_scalar(out[:], xt[:], rstd[:, 0:1],
                                        negmr[:, 0:1], op0=ALU.mult,
                                        op1=ALU.add)
                return rstd, negmr

            # =========== phase A: LN1 + QKV + RoPE ===========
            # software-pipelined: LN of tile t+1 is emitted before the QKV
            # stage of tile t, so each engine's in-order queue matches the
            # dataflow and no engine idles waiting on a later-queued op.
            with tc.tile_pool(name="p_qkv", bufs=4) as pqkv:
                def load_x(t):
                    xt = pqkv.tile([128, D], F32, tag="x_t")
                    nc.sync.dma_start(xt[:], x_d[t * 128:(t + 1) * 128, :])
                    return xt

                def ln_stage(t, xt):
                    n_t = pqkv.tile([128, D], F32R, tag="n_t")
                    ln_norm(pqkv, xt, n_t, "ln1")
                    return n_t

                def qkv_stage(t, n_t):
                    pq = psA.tile([128, 512], F32, tag="pA")
                    nc.tensor.matmul(pq[:, :384], ones1[:], bqkv_t[:],
                                     start=True, stop=False)
                    # transposes run 3 deep ahead of the consuming matmuls
                    hTks = []

                    def emit_transpose(k):
                        pt = psB.tile([128, 128], F32R, tag="pB")
                        nc.tensor.transpose(pt[:],
                                            n_t[:, k * 128:(k + 1) * 128],
                                            ident_r[:])
                        hTk = pqkv.tile([128, 128], F32R, tag="hTk")
                        nc.scalar.copy(hTk[:], pt[:])
                        hTks.append(hTk)

                    for k in range(3):
                        emit_transpose(k)
                    for k in range(8):
                        if k + 3 < 8:
                            emit_transpose(k + 3)
                        nc.tensor.matmul(pq[:, :384], hTks[k][:],
                                         wqkv_sb[k][:],
                                         start=False, stop=(k == 7))
                    # RoPE on q,k (cols 0:256), v copy (cols 256:384)
                    qk = pqkv.tile([128, 256], F32R, tag="qk_rot")
                    vv = pq[:, 0:256].rearrange("p (g u d) -> p g u d",
                                                g=4, u=2, d=HALF)
                    x1 = vv[:, :, 0, :]
                    x2 = vv[:, :, 1, :]
                    ov = qk[:].rearrange("p (g u d) -> p g u d",
                                         g=4, u=2, d=HALF)
                    o1 = ov[:, :, 0, :]
                    o2 = ov[:, :, 1, :]
                    cosb = cos_t[:, t * HALF:(t + 1) * HALF].rearrange(
                        "p (g d) -> p g d", g=1).to_broadcast([128, 4, HALF])
                    sinb = sin_t[:, t * HALF:(t + 1) * HALF].rearrange(
                        "p (g d) -> p g d", g=1).to_broadcast([128, 4, HALF])
                    tA = pqkv.tile([128, 4, HALF], F32, tag="ropeA")
                    tBt = pqkv.tile([128, 4, HALF], F32, tag="ropeB")
                    nc.vector.tensor_tensor(o1, x1, cosb, op=ALU.mult)
                    nc.vector.tensor_tensor(tA[:], x2, sinb, op=ALU.mult)
                    nc.vector.tensor_tensor(o1, o1, tA[:], op=ALU.subtract)
                    nc.vector.tensor_tensor(o2, x2, cosb, op=ALU.mult)
                    nc.vector.tensor_tensor(tBt[:], x1, sinb, op=ALU.mult)
                    nc.vector.tensor_tensor(o2, o2, tBt[:], op=ALU.add)
                    nc.vector.tensor_copy(v_tm[t][:], pq[:, 256:384])
                    # transpose q,k chunks into qT/kT
                    ptq = psB.tile([128, 128], F32R, tag="pB")
                    nc.tensor.transpose(ptq[:], qk[:, 0:128], ident_r[:])
                    nc.scalar.copy(qT[:, t * 128:(t + 1) * 128], ptq[:])
                    ptk = psB.tile([128, 128], F32R, tag="pB")
                    nc.tensor.transpose(ptk[:], qk[:, 128:256], ident_r[:])
                    nc.scalar.copy(kT[:, t * 128:(t + 1) * 128], ptk[:])

                xts = [load_x(0), load_x(1), load_x(2)]
                nts = [ln_stage(0, xts[0])]
                for t in range(NT):
                    if t + 3 < NT:
                        xts.append(load_x(t + 3))
                    if t + 1 < NT:
                        nts.append(ln_stage(t + 1, xts[t + 1]))
                    qkv_stage(t, nts[t])



            # =========== phase B: attention + inline proj ===========
            psB_cm.__exit__(None, None, None)
            psA_cm.__exit__(None, None, None)
            with tc.tile_pool(name="p_att", bufs=5) as patt, \
                 tc.tile_pool(name="psS", bufs=3, space="PSUM") as psS, \
                 tc.tile_pool(name="psT", bufs=3, space="PSUM") as psT, \
                 tc.tile_pool(name="psP", bufs=1, space="PSUM") as psAO, \
                 tc.tile_pool(name="psJ", bufs=1, space="PSUM") as psJ:
                def scores_softmax(b, qi):
                    S = qi + 1
                    W = S * 128
                    qcol = b * T + qi * 128
                    scol = b * T
                    attns = []
                    nch = (W + 511) // 512
                    for hl in range(2):
                        hr = slice(hl * 64, hl * 64 + 64)
                        # scores in per-bank PSUM chunks: exp of chunk 0
                        # overlaps the matmul of chunk 1, and the bank is
                        # released as soon as its exp is done.
                        attn = patt.tile([128, 1024], BF16, tag="attn")
                        sumes = []
                        for ch in range(nch):
                            n0 = ch * 512
                            n1 = min(W, n0 + 512)
                            sc = psS.tile([128, 512], F32, tag="psS")
                            has_diag = qi * 128 >= n0 and qi * 128 < n1
                            nc.tensor.matmul(
                                sc[:, :n1 - n0],
                                qT[hr, qcol:qcol + 128],
                                kT[hr, scol + n0:scol + n1],
                                start=True, stop=not has_diag)
                            if has_diag:
                                # causal mask accumulated on the PE via an
                                # identity matmul: drops a DVE op and a
                                # cross-engine hop from the per-iter chain
                                d0 = qi * 128 - n0
                                nc.tensor.matmul(
                                    sc[:, d0:d0 + 128],
                                    ident_r[:], causal_r[:],
                                    start=False, stop=True)
                            # scores are O(few) at this model scale, so
                            # the softmax max-subtraction is unneeded:
                            # exp() cannot overflow; masked lanes -1e30.
                            sume = patt.tile([128, 2], F32, tag="sume")
                            nc.scalar.activation(attn[:, n0:n1],
                                                 sc[:, :n1 - n0],
                                                 ACTF.Exp,
                                                 bias=z1_t[:, 0:1],
                                                 scale=1.0,
                                                 accum_out=sume[:, 0:1])
                            sumes.append(sume)
                        if nch == 2:
                            nc.vector.tensor_tensor(
                                sumes[0][:, 0:1], sumes[0][:, 0:1],
                                sumes[1][:, 0:1], op=ALU.add)
                        rec = patt.tile([128, 1], F32, tag="rec")
                        nc.vector.reciprocal(rec[:], sumes[0][:, 0:1])
                        nc.gpsimd.tensor_scalar(attn[:, :W], attn[:, :W],
                                                rec[:, 0:1], None,
                                                op0=ALU.mult)
                        attns.append(attn)
                    return attns

                def attnv_proj(b, qi, attns):
                    S = qi + 1
                    qcol = b * T + qi * 128
                    pao = psAO.tile([128, 128], F32, tag="pao")
                    # flatten (hl, si) and emit transposes 2 deep ahead
                    # of the consuming matmuls
                    flat = [(hl, si) for hl in range(2) for si in range(S)]
                    atTs = []

                    def emit_at(j):
                        hl, si = flat[j]
                        pat = psT.tile([128, 128], BF16, tag="pT")
                        nc.tensor.transpose(
                            pat[:],
                            attns[hl][:, si * 128:(si + 1) * 128],
                            ident_bf[:])
                        att_T = patt.tile([128, 128], BF16, tag="attnT")
                        if j % 3 == 2:
                            nc.scalar.copy(att_T[:], pat[:])
                        else:
                            nc.vector.tensor_copy(att_T[:], pat[:])
                        atTs.append(att_T)

                    for j in range(min(2, len(flat))):
                        emit_at(j)
                    for j, (hl, si) in enumerate(flat):
                        if j + 2 < len(flat):
                            emit_at(j + 2)
                        hr = slice(hl * 64, hl * 64 + 64)
                        nc.tensor.matmul(
                            pao[hl * 64:hl * 64 + 64, :],
                            v_tm[b * 8 + si][:, hr],
                            atTs[j][:], start=(si == 0),
                            stop=(si == S - 1))
                    nc.vector.tensor_copy(aoT[:, qcol:qcol + 128], pao[:])
                    # proj for this token tile (both heads done)
                    for nn_ in range(2):
                        pp = psJ.tile([128, 512], F32, tag="pJ")
                        nc.tensor.matmul(pp[:],
                                         aoT[:, qcol:qcol + 128],
                                         wproj_sb[nn_][:], start=True,
                                         stop=True)
                        ps_sb = patt.tile([128, 512], F32, tag="proj_sb")
                        if nn_ == 0:
                            nc.vector.tensor_copy(ps_sb[:], pp[:])
                        else:
                            nc.scalar.copy(ps_sb[:], pp[:])
                        nc.sync.dma_start(
                            prs_in[qcol:qcol + 128,
                                   nn_ * 512:(nn_ + 1) * 512], ps_sb[:])

                # software-pipelined: softmax of iteration i+1 is emitted
                # before the attn@V/proj of iteration i. Largest-W tiles
                # first so the trailing tile (which gates the ReduceScatter)
                # is the cheapest one.
                order = [(b, qi) for qi in range(7, -1, -1)
                         for b in range(B)]
                pend = scores_softmax(*order[0])
                for i, (b, qi) in enumerate(order):
                    nxt = None
                    if i + 1 < len(order):
                        nxt = scores_softmax(*order[i + 1])
                    attnv_proj(b, qi, pend)
                    pend = nxt
            p_attres_cm.__exit__(None, None, None)
            # PSUM pools for router/routing/FFN phases
            psA_cm = tc.tile_pool(name="psA2", bufs=2, space="PSUM")
            psA = psA_cm.__enter__()
            psB_cm = tc.tile_pool(name="psB2", bufs=4, space="PSUM")
            psB = psB_cm.__enter__()
            # FFN w2 (bf16) becomes resident once attention SBUF is freed;
            # loads are emitted after the AllGather kickoff below.
            p_ffnw_cm = tc.tile_pool(name="p_ffnw", bufs=1)
            p_ffnw = p_ffnw_cm.__enter__()
            w2_sb = [p_ffnw.tile([128, D], BF16, tag=f"w2r{m}",
                                 name=f"w2sb{m}") for m in range(DFF // 128)]
            # x_slice tiles loaded ahead of the ReduceScatter
            p_mid_cm = tc.tile_pool(name="p_mid", bufs=1)
            p_mid = p_mid_cm.__enter__()
            xs_l = []
            for i in range(NT_LOC):
                xs = p_mid.tile([128, D], F32, tag=f"xs{i}")
                nc.sync.dma_start(xs[:], xsl_d[i * 128:(i + 1) * 128, :])
                xs_l.append(xs)
            nc.gpsimd.collective_compute(
                "ReduceScatter", ALU.add, replica_groups=RG,
                ins=[prs_in[:]], outs=[prs_out[:]])

            # x_mid = prs_out + x_slice; router logits computed directly
            # from x_mid (logits = rstd*(xm@w') + (-mean*rstd)*colsum(w') +
            # b), so the router matmul does not wait for the LN2 statistics.
            n2_tiles = []
            xm_tiles = []
            for i in range(NT_LOC):
                xs = xs_l[i]
                pr = scr.tile([128, D], F32, tag="misc")
                xm = p_mid.tile([128, D], F32, tag=f"xmid{i}",
                                name=f"xmid{i}")
                # half-width loads+adds so the first router transposes and
                # LN2 stats can start before the second half arrives
                for hh in range(2):
                    cs_ = slice(hh * 512, (hh + 1) * 512)
                    nc.sync.dma_start(pr[:, cs_],
                                      prs_out[i * 128:(i + 1) * 128, cs_])
                    nc.vector.tensor_tensor(xm[:, cs_], pr[:, cs_],
                                            xs[:, cs_], op=ALU.add)
                nc.sync.dma_start(xmid_d[i * 128:(i + 1) * 128, :], xm[:])
                xm_tiles.append(xm)

            with tc.tile_pool(name="p_rout", bufs=2) as prt:
                plg = psB.tile([16, 256], F32, tag="pB")
                for k in range(8):
                    pt = psB.tile([128, 128], F32, tag="pB")
                    h2Tk = prt.tile([128, NT_LOC * 128], F32R, tag="h2T")
                    for i in range(NT_LOC):
                        nc.tensor.transpose(
                            pt[:], xm_tiles[i][:, k * 128:(k + 1) * 128],
                            ident[:])
                        nc.scalar.copy(h2Tk[:, i * 128:(i + 1) * 128], pt[:])
                        pt = psB.tile([128, 128], F32, tag="pB")
                    nc.tensor.matmul(plg[:], wrl_sb[k][:], h2Tk[:],
                                     start=(k == 0), stop=(k == 7))
                # LN2 stats run concurrently with the router matmul
                rs_nm = []
                for i in range(NT_LOC):
                    n2 = p_mid.tile([128, D], F32R, tag=f"n2_{i}",
                                    name=f"n2s{i}")
                    rs_nm.append(ln_norm(p_mid, xm_tiles[i], n2,
                                         f"ln2_{i}"))
                    n2_tiles.append(n2)
                lg_sb = prt.tile([16, 256], F32, tag="lg_sb")
                nc.scalar.copy(lg_sb[:], plg[:])
                lgtm_l = []
                for i in range(NT_LOC):
                    plt = psB.tile([128, 16], F32, tag="pB")
                    nc.tensor.transpose(plt[:],
                                        lg_sb[:, i * 128:(i + 1) * 128],
                                        ident[:16, :16])
                    lraw = prt.tile([128, 16], F32, tag="lraw")
                    nc.scalar.copy(lraw[:], plt[:])
                    rstd_i, negmr_i = rs_nm[i]
                    lgtm = prt.tile([128, 16], F32, tag="lgtm")
                    nc.vector.tensor_scalar(lgtm[:], lraw[:],
                                            rstd_i[:, 0:1], None,
                                            op0=ALU.mult)
                    nc.vector.scalar_tensor_tensor(lgtm[:], csumB[:, 0:16],
                                                   negmr_i[:, 0:1], lgtm[:],
                                                   op0=ALU.mult, op1=ALU.add)
                    nc.vector.tensor_tensor(lgtm[:], lgtm[:], brlB[:, 0:16],
                                            op=ALU.add)
                    lgtm_l.append(lgtm)
                    nc.sync.dma_start(lgag_in[i * 128:(i + 1) * 128, :],
                                      lgtm[:])
                nc.gpsimd.collective_compute(
                    "AllGather", ALU.bypass, replica_groups=RG,
                    ins=[lgag_in[:]], outs=[lgag[:]])
                # h2 payload is derived through (n2 + logit) - logit: a
                # numerically-neutral detour (error ~1 ulp, payload is bf16
                # anyway) that makes the big h2 AllGather depend on the
                # router output, so it cannot be scheduled before the small
                # logits AllGather that routing is waiting on.
                for i in range(NT_LOC):
                    n2b = p_mid.tile([128, D], BF16, tag=f"n2b_{i}")
                    nc.vector.tensor_scalar(n2b[:], n2_tiles[i][:],
                                            lgtm_l[i][:, 0:1],
                                            lgtm_l[i][:, 0:1],
                                            op0=ALU.add, op1=ALU.subtract)
                    nc.sync.dma_start(h2ag_in[i * 128:(i + 1) * 128, :],
                                      n2b[:])
            p_mid_cm.__exit__(None, None, None)
            # zero comb (combine buffer incl. dump row) and meta defaults —
            # emitted here so the DMA traffic lands in the AllGather window
            # instead of competing with phase A input loads.
            with tc.tile_pool(name="p_init", bufs=1) as pinit:
                zrow_bf = pinit.tile([128, D], BF16)
                nc.vector.memset(zrow_bf[:], 0.0)
                for i in range(NT):
                    nc.sync.dma_start(comb[i * 128:(i + 1) * 128, :],
                                      zrow_bf[:])
                nc.sync.dma_start(comb[NTOK:NTOK + 1, :], zrow_bf[:1, :])
                mrow = pinit.tile([128, 2], F32)
                nc.vector.memset(mrow[:, 0:1], 0.0)
                nc.vector.tensor_copy(mrow[:, 1:2], c2048[:])
                for i in range(CAP // 128):
                    nc.sync.dma_start(meta_d[i * 128:(i + 1) * 128, :],
                                      mrow[:])
            nc.gpsimd.collective_compute(
                "AllGather", ALU.bypass, replica_groups=RG,
                ins=[h2ag_in[:]], outs=[h2ag[:]])

            # ---------------- routing (replicated; overlaps h2 AG) --------
            # Two independent passes. Pass 1 computes per-tile top-2 masks,
            # gates and per-tile expert counts; the cross-tile rank offsets
            # are then ONE strict-upper-triangular [16,16] matmul over the
            # stacked counts (no serial offs chain ping-ponging PE<->DVE).
            with tc.tile_pool(name="p_disp", bufs=8) as pdsp:
                mga = resg.tile([128, NT, NEXP], F32R, tag="mga",
                                name="mga")
                gma = resg.tile([128, NT], F32, tag="gma", name="gma")
                mma = resg.tile([128, NT], F32, tag="mma", name="mma")
                pcs_all = psA.tile([1, NT * NEXP], F32, tag="pcs")
                for t in range(NT):
                    lgt = pdsp.tile([128, 16], F32, tag="lgt")
                    nc.sync.dma_start(lgt[:], lgag[t * 128:(t + 1) * 128, :])
                    nzt = pdsp.tile([128, NEXP], F32, tag="nzt")
                    nc.sync.dma_start(nzt[:],
                                      noise_d[t * 128:(t + 1) * 128, :])
                    spu = pdsp.tile([128, NEXP], F32, tag="spu")
                    nc.scalar.activation(spu[:], lgt[:, 8:16], ACTF.Abs,
                                         bias=z1_t[:, 0:1])
                    spe = pdsp.tile([128, NEXP], F32, tag="spe")
                    nc.scalar.activation(spe[:], spu[:], ACTF.Exp,
                                         bias=z1_t[:, 0:1], scale=-1.0)
                    spl = pdsp.tile([128, NEXP], F32, tag="spl")
                    nc.scalar.activation(spl[:], spe[:], ACTF.Ln,
                                         bias=one_t[:, 0:1], scale=1.0)
                    spr = pdsp.tile([128, NEXP], F32, tag="spr")
                    nc.scalar.activation(spr[:], lgt[:, 8:16], ACTF.Relu,
                                         bias=z1_t[:, 0:1])
                    sp = pdsp.tile([128, NEXP], F32, tag="sp")
                    nc.vector.tensor_tensor(sp[:], spl[:], spr[:], op=ALU.add)
                    noisy = pdsp.tile([128, NEXP], F32, tag="noisy")
                    nc.vector.tensor_tensor(noisy[:], nzt[:], sp[:],
                                            op=ALU.mult)
                    nc.vector.tensor_tensor(noisy[:], noisy[:], lgt[:, 0:8],
                                            op=ALU.add)
                    top8 = pdsp.tile([128, 8], F32, tag="top8")
                    nc.vector.max(out=top8[:], in_=noisy[:])
                    v1 = top8[:, 0:1]; v2 = top8[:, 1:2]
                    maskge = mga[:, t, :]
                    nc.vector.tensor_scalar(maskge, noisy[:], v2, None,
                                            op0=ALU.is_ge)
                    eq1 = pdsp.tile([128, NEXP], F32, tag="eq1")
                    nc.vector.tensor_scalar(eq1[:], noisy[:], v1, None,
                                            op0=ALU.is_equal)
                    d21 = pdsp.tile([128, 1], F32, tag="d21")
                    nc.vector.tensor_tensor(d21[:], v2, v1, op=ALU.subtract)
                    e21 = pdsp.tile([128, 1], F32, tag="e21")
                    nc.scalar.activation(e21[:], d21[:], ACTF.Exp,
                                         bias=z1_t[:, 0:1])
                    den = pdsp.tile([128, 1], F32, tag="den")
                    nc.vector.tensor_scalar(den[:], e21[:], 1.0, None,
                                            op0=ALU.add)
                    p1 = pdsp.tile([128, 1], F32, tag="p1")
                    nc.vector.reciprocal(p1[:], den[:])
                    p2 = pdsp.tile([128, 1], F32, tag="p2")
                    nc.vector.tensor_scalar(p2[:], p1[:], -1.0, 1.0,
                                            op0=ALU.mult, op1=ALU.add)
                    p1m2 = pdsp.tile([128, 1], F32, tag="p1m2")
                    nc.scalar.activation(p1m2[:], p1[:], ACTF.Identity,
                                         bias=neg1_t[:, 0:1], scale=2.0)
                    gmask = pdsp.tile([128, NEXP], F32, tag="gmask")
                    nc.vector.tensor_scalar(gmask[:], maskge, p2[:, 0:1],
                                            None, op0=ALU.mult)
                    gate = pdsp.tile([128, NEXP], F32, tag="gate")
                    nc.vector.scalar_tensor_tensor(gate[:], eq1[:],
                                                   p1m2[:, 0:1], gmask[:],
                                                   op0=ALU.mult, op1=ALU.add)
                    # per-tile count of tokens routed to each expert
                    nc.tensor.matmul(
                        pcs_all[0:1, t * NEXP:(t + 1) * NEXP], ones128[:],
                        maskge, start=True, stop=True)
                    # my expert's mask / gate for this tile
                    tsel = pdsp.tile([128, NEXP], F32, tag="tsel")
                    nc.vector.tensor_tensor(tsel[:], maskge, ohB[:, 0:8],
                                            op=ALU.mult)
                    nc.vector.tensor_reduce(mma[:, t:t + 1], tsel[:],
                                            axis=AX.X, op=ALU.add)
                    nc.vector.tensor_tensor(tsel[:], gate[:], ohB[:, 0:8],
                                            op=ALU.mult)
                    nc.vector.tensor_reduce(gma[:, t:t + 1], tsel[:],
                                            axis=AX.X, op=ALU.add)
                # all cross-tile offsets at once: transpose the count row
                # to a column, then one matmul with the host prefix matrix
                # pfx[j,i] = (tile(j) < tile(i)) & (exp(j) == exp(i))
                cs_row = pdsp.tile([1, NT * NEXP], F32, tag="cs_row")
                nc.vector.tensor_copy(cs_row[:], pcs_all[0:1, :])
                pcol = psB.tile([128, 128], F32, tag="pB")
                nc.tensor.transpose(pcol[:, 0:1], cs_row[:],
                                    ident[:1, :1])
                ccol = pdsp.tile([128, 1], F32, tag="ccol")
                nc.vector.tensor_copy(ccol[:], pcol[:, 0:1])
                poffr = psB.tile([128, 128], F32, tag="pB")
                nc.tensor.matmul(poffr[0:1, :], ccol[:], pfx_t[:],
                                 start=True, stop=True)
                offs_row = resg.tile([1, NT * NEXP], F32R, tag="offs_row",
                                     name="offs_row")
                nc.scalar.copy(offs_row[:], poffr[0:1, :])
                for t in range(NT):
                    # rank = SUT.T @ maskge + offs[t] (broadcast)
                    prk = psB.tile([128, NEXP], F32, tag="pB")
                    nc.tensor.matmul(prk[:], sut_t[:], mga[:, t, :],
                                     start=True, stop=False)
                    nc.tensor.matmul(
                        prk[:], ones1[:],
                        offs_row[0:1, t * NEXP:(t + 1) * NEXP],
                        start=False, stop=True)
                    tsel = pdsp.tile([128, NEXP], F32, tag="tsel2")
                    r_me = pdsp.tile([128, 1], F32, tag="r_me")
                    nc.vector.tensor_tensor(tsel[:], prk[:], ohB[:, 0:8],
                                            op=ALU.mult)
                    nc.vector.tensor_reduce(r_me[:], tsel[:], axis=AX.X,
                                            op=ALU.add)
                    # slot = (r_me - 4096)*m_me + 4096
                    slotf = pdsp.tile([128, 1], F32, tag="slotf")
                    nc.vector.scalar_tensor_tensor(slotf[:], r_me[:], -4096.0,
                                                   mma[:, t:t + 1],
                                                   op0=ALU.add,
                                                   op1=ALU.mult)
                    nc.vector.tensor_scalar(slotf[:], slotf[:], 4096.0, None,
                                            op0=ALU.add)
                    slot_i = pdsp.tile([128, 1], I32, tag="slot_i")
                    nc.vector.tensor_copy(slot_i[:], slotf[:])
                    # meta row (gate, token id) scattered by slot
                    meta = pdsp.tile([128, 2], F32, tag="meta")
                    nc.vector.tensor_copy(meta[:, 0:1], gma[:, t:t + 1])
                    nc.vector.tensor_copy(meta[:, 1:2], rowid_t[:, t:t + 1])
                    nc.gpsimd.indirect_dma_start(
                        out=meta_d[:],
                        out_offset=IndirectOffsetOnAxis(ap=slot_i[:], axis=0),
                        in_=meta[:], in_offset=None,
                        bounds_check=CAP - 1, oob_is_err=False)

            # FFN weight preloads land in the AllGather window. The w1 DMAs
            # are anchored behind the logits AG via a tiny WAW write so the
            # serialized DMA device is not stolen from phase A x-tile loads.
            anchor = resg.tile([1, 1], F32, tag="anchor")
            nc.sync.dma_start(anchor[:], lgag[0:1, 0:1])
            anchor_b = resg.tile([1, 1], BF16, tag="anchorb")
            nc.vector.tensor_copy(anchor_b[:], anchor[:])
            for k in range(8):
                nc.vector.tensor_copy(w1_sb[k][0:1, 0:1], anchor_b[:])
                nc.sync.dma_start(w1_sb[k][:],
                                  w1_d[k * 128:(k + 1) * 128, :])
            # w2 preload into the RS/AllGather window
            for m in range(DFF // 128):
                nc.sync.dma_start(w2_sb[m][:],
                                  w2_d[m * 128:(m + 1) * 128, :])

            # ---------------- expert FFN ----------------
            sel_i = []
            with tc.tile_pool(name="p_ffn", bufs=1) as pffn:
                gb_sb = pffn.tile([128, CAP], F32, tag="gb")
                xeT = []
                for m in range(8):
                    xm_ = pffn.tile([128, CAP], BF16, tag=f"xeT{m}",
                                    name=f"xeT{m}")
                    xeT.append(xm_)
                with tc.tile_pool(name="p_xe", bufs=1) as pxe:
                    # gather xe rows (bf16) by slot->token map from meta
                    xe_sb = []
                    for c in range(CAP // 128):
                        xs_ = pxe.tile([128, D], BF16, tag=f"xe{c}",
                                       name=f"xe_sb{c}")
                        nc.vector.memset(xs_[:], 0.0)
                        xe_sb.append(xs_)
                    grow = pxe.tile([1, CAP], F32R, tag="grow")
                    for c in range(CAP // 128):
                        mt = scr.tile([128, 2], F32, tag="mt")
                        nc.sync.dma_start(mt[:],
                                          meta_d[c * 128:(c + 1) * 128, :])
                        si_ = resg.tile([128, 1], I32, tag=f"sel{c}",
                                        name=f"sel_i{c}")
                        nc.vector.tensor_copy(si_[:], mt[:, 1:2])
                        sel_i.append(si_)
                        nc.gpsimd.indirect_dma_start(
                            out=xe_sb[c][:],
                            out_offset=None,
                            in_=h2ag[:],
                            in_offset=IndirectOffsetOnAxis(ap=si_[:],
                                                           axis=0),
                            bounds_check=NTOK - 1, oob_is_err=False)
                        pgt = psB.tile([128, 128], F32, tag="pB")
                        nc.tensor.transpose(pgt[:1, :], mt[:, 0:1],
                                            ident[:])
                        nc.scalar.copy(grow[0:1, c * 128:(c + 1) * 128],
                                       pgt[:1, :])
                    # gate row -> broadcast [128, CAP]
                    pgb = psA.tile([128, 512], F32, tag="pA")
                    nc.tensor.matmul(pgb[:], ones1[:], grow[:], start=True,
                                     stop=True)
                    nc.scalar.copy(gb_sb[:], pgb[:])
                    # transpose xe -> xeT (bf16)
                    for c in range(CAP // 128):
                        for m in range(8):
                            pt = psB.tile([128, 128], BF16, tag="pB")
                            nc.tensor.transpose(
                                pt[:], xe_sb[c][:, m * 128:(m + 1) * 128],
                                ident_bf[:])
                            if m % 2 == 0:
                                nc.vector.tensor_copy(
                                    xeT[m][:, c * 128:(c + 1) * 128], pt[:])
                            else:
                                nc.scalar.copy(
                                    xeT[m][:, c * 128:(c + 1) * 128], pt[:])
                # y1 = relu(xe @ w1 + b1)  (w1 SBUF-resident bf16)
                y1 = []
                for m in range(DFF // 128):
                    py = psA.tile([128, 512], F32, tag="pA")
                    for k in range(8):
                        nc.tensor.matmul(py[:],
                                         w1_sb[k][:, m * 128:(m + 1) * 128],
                                         xeT[k][:],
                                         start=(k == 0), stop=(k == 7))
                    y1m = pffn.tile([128, CAP], BF16, tag=f"y1_{m}",
                                    name=f"y1m{m}")
                    nc.scalar.activation(y1m[:], py[:], ACTF.Relu,
                                         bias=b1_t[:, m:m + 1], scale=1.0)
                    y1.append(y1m)
                # y2 = (y1 @ w2 + b2) * gate; transpose per-n into oc tiles
                oc_tiles = []
                for c in range(CAP // 128):
                    occ = pffn.tile([128, D], BF16, tag=f"oc{c}",
                                    name=f"oc{c}")
                    oc_tiles.append(occ)
                for n in range(8):
                    py = psA.tile([128, 512], F32, tag="pA")
                    for m in range(DFF // 128):
                        nc.tensor.matmul(py[:],
                                         w2_sb[m][:, n * 128:(n + 1) * 128],
                                         y1[m][:],
                                         start=(m == 0),
                                         stop=(m == DFF // 128 - 1))
                    oTn = pffn.tile([128, CAP], BF16, tag="oTn")
                    nc.vector.scalar_tensor_tensor(oTn[:], py[:],
                                                   b2_t[:, n:n + 1], gb_sb[:],
                                                   op0=ALU.add, op1=ALU.mult)
                    for c in range(CAP // 128):
                        pt = psB.tile([128, 128], BF16, tag="pB")
                        nc.tensor.transpose(pt[:],
                                            oTn[:, c * 128:(c + 1) * 128],
                                            ident_bf[:])
                        if c % 2 == 0:
                            nc.vector.tensor_copy(
                                oc_tiles[c][:, n * 128:(n + 1) * 128], pt[:])
                        else:
                            nc.scalar.copy(
                                oc_tiles[c][:, n * 128:(n + 1) * 128], pt[:])
                for c in range(CAP // 128):
                    nc.gpsimd.indirect_dma_start(
                        out=comb[:],
                        out_offset=IndirectOffsetOnAxis(ap=sel_i[c][:],
                                                        axis=0),
                        in_=oc_tiles[c][:], in_offset=None,
                        bounds_check=NTOK, oob_is_err=False)

            # x_mid reload overlaps the final ReduceScatter
            xmf = []
            for i in range(NT_LOC):
                xm = scr.tile([128, D], F32, tag="misc")
                nc.sync.dma_start(xm[:], xmid_d[i * 128:(i + 1) * 128, :])
                xmf.append(xm)
            nc.gpsimd.collective_compute(
                "ReduceScatter", ALU.add, replica_groups=RG,
                ins=[comb[0:NTOK, :]], outs=[rs2_out[:]])

            for i in range(NT_LOC):
                rt = scr.tile([128, D], BF16, tag="miscb")
                nc.sync.dma_start(rt[:], rs2_out[i * 128:(i + 1) * 128, :])
                nc.vector.tensor_tensor(xmf[i][:], xmf[i][:], rt[:],
                                        op=ALU.add)
                nc.sync.dma_start(out_d[i * 128:(i + 1) * 128, :],
                                  xmf[i][:])
            p_ffnw_cm.__exit__(None, None, None)
            psB_cm.__exit__(None, None, None)
            psA_cm.__exit__(None, None, None)

    split_multiwaits(nc)
    return nc


_NC_CACHE = None


def _get_nc():
    global _NC_CACHE
    if _NC_CACHE is None:
        _NC_CACHE = build_kernel()
    return _NC_CACHE


def _host_inputs(x, noise, ln1_g, ln1_b, ln2_g, ln2_b, w_qkv, w_proj,
                 w_rl, b_rl, w_rn, b_rn, w1, b1, w2, b2):
    f = np.float32
    x_full = np.ascontiguousarray(x.reshape(NTOK, D), f)
    noise_t = np.ascontiguousarray(noise.reshape(NTOK, NEXP), f)
    # RoPE tables (matches reference build_sin_cos)
    pos = np.arange(T, dtype=np.float64)[:, None]
    inv = np.exp(np.arange(0, DH, 2, dtype=np.float64) *
                 (-math.log(10000.0) / DH))
    ang = pos * inv   # (T, 32)
    sin_full = np.sin(ang).astype(f)
    cos_full = np.cos(ang).astype(f)
    cos_tm = np.zeros((128, NT * HALF), f)
    sin_tm = np.zeros((128, NT * HALF), f)
    for t in range(NT):
        g = t * 128 + np.arange(128)
        p_ = g % T
        cos_tm[:, t * HALF:(t + 1) * HALF] = cos_full[p_]
        sin_tm[:, t * HALF:(t + 1) * HALF] = sin_full[p_]
    sut = np.triu(np.ones((128, 128), f), 1)
    qi_ = np.arange(128)[:, None]
    si_ = np.arange(128)[None, :]
    causal = np.where(si_ <= qi_, 0.0, -1e30).astype(f)
    rowid = (np.arange(NT)[None, :] * 128 +
             np.arange(128)[:, None]).astype(f)
    jj = np.arange(128)
    pfx = ((jj[:, None] // NEXP < jj[None, :] // NEXP) &
           (jj[:, None] % NEXP == jj[None, :] % NEXP)).astype(f)
    # fold ln2 gamma/beta into router + expert up-proj weights
    g1 = np.asarray(ln1_g, np.float64)
    b1v = np.asarray(ln1_b, np.float64)
    g2 = np.asarray(ln2_g, np.float64)
    b2v = np.asarray(ln2_b, np.float64)
    w_rlrn_full = np.concatenate([w_rl, w_rn], axis=1).astype(np.float64)
    w_rlrn = (g2[:, None] * w_rlrn_full).astype(f)
    b_rlrn = (np.concatenate([b_rl, b_rn]).astype(np.float64) +
              b2v @ w_rlrn_full).reshape(1, 16).astype(f)
    csum_rlrn = w_rlrn.astype(np.float64).sum(axis=0).reshape(1, 16).astype(f)

    in_maps = []
    for c in range(NC):
        h0 = 2 * c
        qcols = slice(h0 * DH, h0 * DH + 128)
        wq = w_qkv[:, 0:D][:, qcols] * (1.0 / math.sqrt(DH))
        wk = w_qkv[:, D:2 * D][:, qcols]
        wv = w_qkv[:, 2 * D:3 * D][:, qcols]
        w_qkv_cols = np.concatenate([wq, wk, wv], axis=1).astype(np.float64)
        w_qkv_l = (g1[:, None] * w_qkv_cols).astype(f)
        b_qkv_l = (b1v @ w_qkv_cols).reshape(1, 384).astype(f)
        onehot = np.zeros((1, NEXP), f)
        onehot[0, c] = 1.0
        w1c = np.asarray(w1[c], np.float64)
        w1_l = (g2[:, None] * w1c).astype(ml_dtypes.bfloat16)
        b1_l = (np.asarray(b1[c], np.float64) + b2v @ w1c).astype(f)
        m = {
            "x_full": x_full,
            "x_slice": x_full[c * LT:(c + 1) * LT],
            "cos_tm": cos_tm, "sin_tm": sin_tm,
            "w_qkv_l": np.ascontiguousarray(w_qkv_l),
            "b_qkv_l": np.ascontiguousarray(b_qkv_l),
            "w_proj_l": np.ascontiguousarray(w_proj[c * 128:(c + 1) * 128, :], f),
            "w_rlrn": w_rlrn,
            "b_rlrn": b_rlrn,
            "csum_rlrn": csum_rlrn,
            "w1_l": np.ascontiguousarray(w1_l),
            "w2_l": np.ascontiguousarray(w2[c].astype(ml_dtypes.bfloat16)),
            "b1_l": np.ascontiguousarray(b1_l.reshape(DFF // 128, 128).T, f),
            "b2_l": np.ascontiguousarray(b2[c].reshape(D // 128, 128).T, f),
            "noise_t": noise_t,
            "onehot": onehot,
            "rowid": rowid,
            "pfx": pfx,
            "sut": sut,
            "causal": causal,
        }
        in_maps.append(m)
    return in_maps


def kernel(**inputs):
    nc = _get_nc()
    in_maps = _host_inputs(**{k: np.asarray(v) for k, v in inputs.items()})
    res = run_bass_kernel_spmd(nc, in_maps, core_ids=list(range(NC)))
    out = np.concatenate([res.results[c]["out_c"] for c in range(NC)], axis=0)
    return out.reshape(B, T, D).astype(np.float32)


if __name__ == "__main__":
    nc = build_kernel()
    ni = sum(len(bb.instructions) for fn in nc.m.functions for bb in fn.blocks)
    print("built ok, instructions:", ni)
